# revision 1
# baseline (speedup 1.0000x reference)
"""Trainium2 Bass kernel for nn_BoostEnhancedAttention.

Reference computation:
    v   = (values @ W_v.T + b_v)                      # [B, NK, H*D_V]
    att = softmax(att3 ⊗ att12 interleaved, axis=k)   # [B, H, NQ, NK]
    out = (att @ v_per_head) @ W_o.T + b_o            # [B, NQ, D_MODEL]

Restructuring used here (exact algebra, verified vs reference):
  - Scores factor as s[b,h,q,k] = att3[b,h,q,c(k)] * att12[b,h,...f(k)], so
    exp(s) is computed by the ACT engine directly with the multiply folded
    into the activation's per-partition `scale` operand. No separate score
    build pass.
  - Since softmax rows sum to 1 and both projections are linear, fold
    W_v/W_o into per-head M_h = W_o[:,h] @ W_v[h,:] and apply AFTER
    attention:  out[b] = sum_h (att_h @ values[b]) @ M_h.T + b_eff.
    This lets the attention matmul consume `values` in natural [k, d]
    layout (k on partitions) — no transpose of the big tensor anywhere.
  - Softmax normalization deferred: G~ = E @ values accumulated
    unnormalized in PSUM; Z = column sums of E obtained with a ones-matmul
    (output replicated across all 128 partitions so the normalizing
    multiply needs no partition broadcast).

Sharding: data-parallel over batch, B=32 over 8 cores -> 4 batches/core.
No collectives needed; outputs concatenated on host.
"""

import numpy as np
import ml_dtypes

B, CH, CW, H, FH, FW = 32, 16, 16, 8, 4, 4
NQ = 64
NCELL = CH * CW          # 256 coarse cells (c)
F = FH * FW              # 16 fine positions per cell
NK = NCELL * F           # 4096
D_IN, D_V, D_MODEL = 512, 64, 512
N_CORES = 8
B_LOC = B // N_CORES     # 4
N_KT = 32                # k-tiles of 128: kt = half*16 + f, partition = c_loc
N_DT = 4                 # d_in tiles of 128
HQ = H * NQ              # 512

BF16 = ml_dtypes.bfloat16


def _k_perm():
    """perm[k'] -> original k, where k' = (half*16+f)*128 + c_loc.

    Original key order is (ch, fh, cw, fw):  k = ch*256 + fh*64 + cw*4 + fw.
    New order groups a k-tile as (fixed f=(fh,fw), c = half*128 + c_loc).
    """
    perm = np.zeros(NK, np.int64)
    c = np.arange(NCELL)
    ch_i, cw_i = c // CW, c % CW
    for half in range(2):
        for f in range(F):
            kt = half * F + f
            fh, fw = f // FW, f % FW
            cc = half * 128 + np.arange(128)
            perm[kt * 128:(kt + 1) * 128] = (
                ch_i[cc] * (FH * CW * FW) + fh * (CW * FW) + cw_i[cc] * FW + fw
            )
    return perm


_PERM = _k_perm()
_NC_CACHE = {}


def _build_nc():
    from contextlib import ExitStack

    import concourse.bass as bass
    import concourse.tile as tile
    from concourse import bacc, mybir

    f32 = mybir.dt.float32
    bf16 = mybir.dt.bfloat16

    nc = bacc.Bacc("TRN2", target_bir_lowering=False, debug=False,
                   num_devices=N_CORES)

    values_r = nc.dram_tensor("values_r", [B_LOC, NK, D_IN], bf16,
                              kind="ExternalInput")
    att3_t = nc.dram_tensor("att3_t", [B_LOC, NCELL, HQ], bf16,
                            kind="ExternalInput")
    att12_pair = nc.dram_tensor("att12_pair", [B_LOC, NCELL, F * H * 2], bf16,
                                kind="ExternalInput")
    m_all = nc.dram_tensor("m_all", [128, N_DT * H * D_MODEL], bf16,
                           kind="ExternalInput")
    beff = nc.dram_tensor("beff", [1, D_MODEL], bf16, kind="ExternalInput")
    out = nc.dram_tensor("out", [B_LOC * NQ, D_MODEL], f32,
                         kind="ExternalOutput")

    with tile.TileContext(nc) as tc, ExitStack() as ctx:
        const_pool = ctx.enter_context(tc.tile_pool(name="const", bufs=1))
        a3_pool = ctx.enter_context(tc.tile_pool(name="a3", bufs=2))
        a12r_pool = ctx.enter_context(tc.tile_pool(name="a12r", bufs=2))
        vt_pool = ctx.enter_context(tc.tile_pool(name="vt", bufs=20))
        sc_pool = ctx.enter_context(tc.tile_pool(name="sc", bufs=4))
        et_pool = ctx.enter_context(tc.tile_pool(name="et", bufs=4))
        esum_pool = ctx.enter_context(tc.tile_pool(name="esum", bufs=2))
        zb_pool = ctx.enter_context(tc.tile_pool(name="zb", bufs=2))
        g_pool = ctx.enter_context(tc.tile_pool(name="gps", bufs=1, space="PSUM"))
        z_pool = ctx.enter_context(tc.tile_pool(name="zps", bufs=1, space="PSUM"))
        o_pool = ctx.enter_context(tc.tile_pool(name="ops", bufs=1, space="PSUM"))
        o_sb_pool = ctx.enter_context(tc.tile_pool(name="osb", bufs=2))

        ones_sb = const_pool.tile([128, 128], bf16)
        nc.vector.memset(ones_sb[:], 1.0)
        warm_sb = const_pool.tile([128, D_MODEL], bf16, name="warm_sb")
        nc.vector.memset(warm_sb[:], 1.0)
        warm = o_pool.tile([128, D_MODEL], f32, tag="o", name="warm")
        for wi in range(12):
            nc.tensor.matmul(warm[:], ones_sb[:], warm_sb[:],
                             start=True, stop=True)
        beff_sb = const_pool.tile([1, D_MODEL], bf16)
        nc.sync.dma_start(beff_sb[:], beff.ap())
        # g_all[d_loc, (dt, h, b, q)] : normalized attention output, bf16
        g_all = const_pool.tile([128, N_DT * H * B_LOC * NQ], bf16)

        Q2 = NQ // 2

        def emit_group(b, half, gi, FQ, f0, a3_t, a12r_t):
            """One score group: broadcast multiply + exp for FQ f-positions."""
            a3b = a3_t[half][:]
            in0 = bass.AP(a3b.tensor, a3b.offset,
                          [a3b.ap[0], [0, FQ], [NQ, H], [2, Q2], [1, 2]])
            sc = sc_pool.tile([128, 4 * HQ], bf16, tag="sc",
                              name=f"sc_{b}_{half}_{gi}")
            scb = sc[:]
            out_ap = bass.AP(scb.tensor, scb.offset,
                             [scb.ap[0], [HQ, FQ], [NQ, H], [2, Q2], [1, 2]])
            a12b = a12r_t[half][:]
            in1 = bass.AP(a12b.tensor, a12b.offset + f0 * H * 2,
                          [a12b.ap[0], [H * 2, FQ], [2, H], [0, Q2], [1, 2]])
            nc.vector.tensor_mul(out_ap, in0, in1)
            et = et_pool.tile([128, 4 * HQ], bf16, tag="et",
                              name=f"et_{b}_{half}_{gi}")
            nc.scalar.activation(et[:, :FQ * HQ], sc[:, :FQ * HQ],
                                 mybir.ActivationFunctionType.Exp)
            return et

        def prologue(b):
            """Input DMAs + first score group for batch b — emitted ahead of
            the previous batch's epilogue so the DVE/ACT pipeline stays
            primed across the batch transition."""
            a3_t = [a3_pool.tile([128, HQ], bf16, tag=f"a3_{hf}",
                                 name=f"a3_{b}_{hf}") for hf in range(2)]
            for hf in range(2):
                nc.sync.dma_start(a3_t[hf][:],
                                  att3_t.ap()[b, hf * 128:(hf + 1) * 128, :])
            a12r_t = []
            for hf in range(2):
                a12r = a12r_pool.tile([128, F * H * 2], bf16, tag=f"a12r_{hf}",
                                      name=f"a12r_{b}_{hf}")
                nc.sync.dma_start(a12r[:],
                                  att12_pair.ap()[b, hf * 128:(hf + 1) * 128, :])
                a12r_t.append(a12r)
            groups = [1, 1, 2, 4, 4, 4] if b == 0 else [4, 4, 4, 4]
            et0 = emit_group(b, 0, 0, groups[0], 0, a3_t, a12r_t)
            return a3_t, a12r_t, groups, et0

        pro = prologue(0)
        for b in range(B_LOC):
            a3_t, a12r_t, groups0, et0 = pro
            gps = [g_pool.tile([128, HQ], f32, tag=f"g{dt}", name=f"g_{b}_{dt}",
                               bufs=(2 if dt < 2 else 1))
                   for dt in range(N_DT)]
            esum = esum_pool.tile([128, HQ], bf16)

            for half in range(2):
                groups = groups0 if half == 0 else [4, 4, 4, 4]
                f0 = 0
                for gi, FQ in enumerate(groups):
                    if half == 0 and gi == 0:
                        et = et0
                    else:
                        et = emit_group(b, half, gi, FQ, f0, a3_t, a12r_t)
                    for j in range(FQ):
                        kt = half * F + f0 + j
                        vt = vt_pool.tile([128, D_IN], bf16, tag="vt",
                                          name=f"vt_{b}_{kt}")
                        nc.sync.dma_start(
                            vt[:], values_r.ap()[b, kt * 128:(kt + 1) * 128, :])
                        ets = et[:, j * HQ:(j + 1) * HQ]
                        DEFER = 5
                        if kt < DEFER:
                            if kt == 0:
                                deferred = []
                            for dt in range(2):
                                nc.tensor.matmul(gps[dt][:],
                                                 vt[:, dt * 128:(dt + 1) * 128],
                                                 ets, start=(kt == 0),
                                                 stop=False)
                            deferred.append((vt, ets, kt == 0))
                            if kt == DEFER - 1:
                                for dvt, dets, dstart in deferred:
                                    for dt in range(2, N_DT):
                                        nc.tensor.matmul(
                                            gps[dt][:],
                                            dvt[:, dt * 128:(dt + 1) * 128],
                                            dets, start=dstart, stop=False)
                        else:
                            for dt in range(N_DT):
                                nc.tensor.matmul(gps[dt][:],
                                                 vt[:, dt * 128:(dt + 1) * 128],
                                                 ets,
                                                 start=False,
                                                 stop=(kt == N_KT - 1))
                        if kt == 0:
                            nc.vector.tensor_copy(esum[:], ets)
                        else:
                            nc.vector.tensor_add(esum[:], esum[:], ets)
                    f0 += FQ
                if b == 1 and half == 0:
                    # weights for the output projection, emitted mid-stream so
                    # the transfer never contends with critical prefetches
                    m_sb = const_pool.tile([128, N_DT * H * D_MODEL], bf16,
                                           name="m_sb")
                    mq = N_DT * H * D_MODEL // 4
                    for mi in range(4):
                        nc.sync.dma_start(m_sb[:, mi * mq:(mi + 1) * mq],
                                          m_all.ap()[:, mi * mq:(mi + 1) * mq])

            if b + 1 < B_LOC:
                pro = prologue(b + 1)

            # Z replicated on all partitions via ones-matmul, then 1/Z
            zps = z_pool.tile([128, HQ], f32, tag="z")
            nc.tensor.matmul(zps[:], ones_sb[:], esum[:], start=True, stop=True)
            zb = zb_pool.tile([128, HQ], f32)
            nc.vector.reciprocal_approx_fast(zb[:], zps[:])

            ga_v = g_all[:].rearrange("p (dt h bb q) -> p dt h bb q",
                                      dt=N_DT, h=H, bb=B_LOC)
            for dt in range(N_DT):
                nc.vector.tensor_mul(
                    ga_v[:, dt, :, b, :],
                    gps[dt][:].rearrange("p (h q) -> p h q", h=H),
                    zb[:].rearrange("p (h q) -> p h q", h=H),
                )

            # Output projection for each completed pair of batches
            if b % 2 == 1:
                bq = b // 2
                ops = o_pool.tile([128, D_MODEL], f32, tag="o")
                i = 0
                for dt in range(N_DT):
                    for h in range(H):
                        col = dt * (H * B_LOC * NQ) + h * (B_LOC * NQ) + bq * 128
                        nc.tensor.matmul(
                            ops[:],
                            g_all[:, col:col + 128],
                            m_sb[:, (dt * H + h) * D_MODEL:
                                 (dt * H + h + 1) * D_MODEL],
                            start=(i == 0), stop=False)
                        i += 1
                # bias via K=1 matmul (broadcasts b_eff to all partitions)
                nc.tensor.matmul(ops[:], ones_sb[0:1, :], beff_sb[:],
                                 start=False, stop=True)
                out_sb = o_sb_pool.tile([128, D_MODEL], f32, tag="osb",
                                        name=f"osb_{bq}")
                nc.vector.tensor_copy(out_sb[:], ops[:])
                nc.sync.dma_start(out.ap()[bq * 128:(bq + 1) * 128, :],
                                  out_sb[:])

    nc.compile()
    return nc


def _get_nc():
    if "nc" not in _NC_CACHE:
        _NC_CACHE["nc"] = _build_nc()
    return _NC_CACHE["nc"]


def _host_prep(att12, att3, values, W_v, b_v, W_o, b_o):
    att12 = np.asarray(att12, np.float32)
    att3 = np.asarray(att3, np.float32)
    values = np.asarray(values, np.float32)
    W_v = np.asarray(W_v, np.float32)
    b_v = np.asarray(b_v, np.float32)
    W_o = np.asarray(W_o, np.float32)
    b_o = np.asarray(b_o, np.float32)

    values_r = np.ascontiguousarray(values[:, _PERM, :]).astype(BF16)
    att3_t = np.ascontiguousarray(
        att3.transpose(0, 3, 1, 2).reshape(B, NCELL, HQ)).astype(BF16)
    att12_r = np.ascontiguousarray(
        att12.transpose(0, 1, 2, 4, 5, 3).reshape(B, NCELL, F * H)).astype(BF16)
    att12_pair = np.ascontiguousarray(np.broadcast_to(
        att12_r[:, :, :, None], (B, NCELL, F * H, 2)).reshape(
        B, NCELL, F * H * 2))

    # Per-head folded projection M_h = W_o_h @ W_v_h  [D_MODEL, D_IN]
    Wv3 = W_v.reshape(H, D_V, D_IN)
    Wo3 = W_o.reshape(D_MODEL, H, D_V)
    M = np.einsum("dhv,hvi->hdi", Wo3, Wv3)          # [H, DM, DIN]
    # m_all[d_loc, (dt, h, dm)] = M[h].T[dt*128+d_loc, dm]
    Mt = M.transpose(0, 2, 1)                        # [H, DIN, DM]
    m_all = np.ascontiguousarray(
        Mt.reshape(H, N_DT, 128, D_MODEL).transpose(2, 1, 0, 3)
        .reshape(128, N_DT * H * D_MODEL)).astype(BF16)

    b_eff = b_o + np.einsum("dhv,hv->d", Wo3, b_v.reshape(H, D_V))
    beff = b_eff.reshape(1, D_MODEL).astype(BF16)
    return values_r, att3_t, att12_pair, m_all, beff


def kernel(att12, att3, values, W_v, b_v, W_o, b_o):
    from concourse.bass_utils import run_bass_kernel_spmd

    values_r, att3_t, att12_pair, m_all, beff = _host_prep(
        att12, att3, values, W_v, b_v, W_o, b_o)

    in_maps = []
    for core in range(N_CORES):
        s = slice(core * B_LOC, (core + 1) * B_LOC)
        in_maps.append({
            "values_r": np.ascontiguousarray(values_r[s]),
            "att3_t": np.ascontiguousarray(att3_t[s]),
            "att12_pair": np.ascontiguousarray(att12_pair[s]),
            "m_all": m_all,
            "beff": beff,
        })

    nc = _get_nc()
    res = run_bass_kernel_spmd(nc, in_maps, core_ids=list(range(N_CORES)))
    out = np.concatenate(
        [res.results[i]["out"].reshape(B_LOC, NQ, D_MODEL)
         for i in range(N_CORES)], axis=0)
    return out.astype(np.float32)



# revision 6
# speedup vs baseline: 1.1072x; 1.1072x over previous
"""Trainium2 Bass kernel for nn_BoostEnhancedAttention.

Reference computation:
    v   = (values @ W_v.T + b_v)                      # [B, NK, H*D_V]
    att = softmax(att3 ⊗ att12 interleaved, axis=k)   # [B, H, NQ, NK]
    out = (att @ v_per_head) @ W_o.T + b_o            # [B, NQ, D_MODEL]

Restructuring used here (exact algebra, verified vs reference):
  - Scores factor as s[b,h,q,k] = att3[b,h,q,c(k)] * att12[b,h,...f(k)]; E =
    exp(s) built per k-tile by DVE multiply + ACT exp.
  - Attention consumes `values` in natural [k, d] layout:
    G~[din, (h,q)] = sum_k V[k, din] E[k, (h,q)] accumulated unnormalized in
    PSUM. Z = column sums of E via ones-matmul (replicated on all partitions).
  - G~ is drained from PSUM to SBUF *unnormalized* (no dependency on Z), so
    PSUM banks free immediately and the next batch's accumulation overlaps
    the entire softmax/projection epilogue.
  - Output projection in two rank-64 stages instead of folded per-head
    M_h = W_o_h @ W_v_h (which costs 8192 PE cycles/batch):
      step1: Y[(h,dv), q]  = W_v_h.T-contraction over din   (2048 cycles)
      normalize: Y *= 1/Z[h,q]  (tiny DVE op on the 8x-smaller Y)
      step2: out[dm, q]    = W_o-contraction over (h,dv)    (1024 cycles)
    Output lands as [dm, (b,q)]; final transpose happens on host.

Sharding: data-parallel over batch, B=32 over 8 cores -> 4 batches/core.
No collectives needed; outputs concatenated on host.
"""

import numpy as np
import ml_dtypes

B, CH, CW, H, FH, FW = 32, 16, 16, 8, 4, 4
NQ = 64
NCELL = CH * CW          # 256 coarse cells (c)
F = FH * FW              # 16 fine positions per cell
NK = NCELL * F           # 4096
D_IN, D_V, D_MODEL = 512, 64, 512
N_CORES = 8
B_LOC = B // N_CORES     # 4
N_KT = 32                # k-tiles of 128: kt = half*16 + f, partition = c_loc
N_DT = 4                 # d_in tiles of 128
HQ = H * NQ              # 512
HP = H // 2              # head pairs (partition-packed in Y)

BF16 = ml_dtypes.bfloat16


def _k_perm():
    """perm[k'] -> original k, where k' = (half*16+f)*128 + c_loc.

    Original key order is (ch, fh, cw, fw):  k = ch*256 + fh*64 + cw*4 + fw.
    New order groups a k-tile as (fixed f=(fh,fw), c = half*128 + c_loc).
    """
    perm = np.zeros(NK, np.int64)
    c = np.arange(NCELL)
    ch_i, cw_i = c // CW, c % CW
    for half in range(2):
        for f in range(F):
            kt = half * F + f
            fh, fw = f // FW, f % FW
            cc = half * 128 + np.arange(128)
            perm[kt * 128:(kt + 1) * 128] = (
                ch_i[cc] * (FH * CW * FW) + fh * (CW * FW) + cw_i[cc] * FW + fw
            )
    return perm


_PERM = _k_perm()
_NC_CACHE = {}


def _build_nc():
    from contextlib import ExitStack

    import concourse.bass as bass
    import concourse.tile as tile
    from concourse import bacc, mybir

    f32 = mybir.dt.float32
    bf16 = mybir.dt.bfloat16
    Copy = mybir.ActivationFunctionType.Copy
    Exp = mybir.ActivationFunctionType.Exp

    nc = bacc.Bacc("TRN2", target_bir_lowering=False, debug=False,
                   num_devices=N_CORES)

    values_r = nc.dram_tensor("values_r", [B_LOC, NK, D_IN], bf16,
                              kind="ExternalInput")
    att3_t = nc.dram_tensor("att3_t", [B_LOC, NCELL, HQ], bf16,
                            kind="ExternalInput")
    att12_pair = nc.dram_tensor("att12_pair", [B_LOC, NCELL, F * H * 2], bf16,
                                kind="ExternalInput")
    wv_all = nc.dram_tensor("wv_all", [128, N_DT * H * D_V], bf16,
                            kind="ExternalInput")
    wo_all = nc.dram_tensor("wo_all", [128, HP * N_DT * 128], bf16,
                            kind="ExternalInput")
    beff_t = nc.dram_tensor("beff_t", [128, N_DT], f32, kind="ExternalInput")
    # out[dmt, dm_loc, b*NQ + q]
    out = nc.dram_tensor("out", [N_DT, 128, B_LOC * NQ], f32,
                         kind="ExternalOutput")

    with tile.TileContext(nc) as tc, ExitStack() as ctx:
        const_pool = ctx.enter_context(tc.tile_pool(name="const", bufs=1))
        a3_pool = ctx.enter_context(tc.tile_pool(name="a3", bufs=2))
        a12r_pool = ctx.enter_context(tc.tile_pool(name="a12r", bufs=2))
        vt_pool = ctx.enter_context(tc.tile_pool(name="vt", bufs=20))
        sc_pool = ctx.enter_context(tc.tile_pool(name="sc", bufs=4))
        et_pool = ctx.enter_context(tc.tile_pool(name="et", bufs=6))
        esum_pool = ctx.enter_context(tc.tile_pool(name="esum", bufs=2))
        zb_pool = ctx.enter_context(tc.tile_pool(name="zb", bufs=2))
        gall_pool = ctx.enter_context(tc.tile_pool(name="gall", bufs=2))
        ysb_pool = ctx.enter_context(tc.tile_pool(name="ysb", bufs=2))
        osb_pool = ctx.enter_context(tc.tile_pool(name="osb", bufs=2))
        # PSUM: 5 (gps) + 1 (z) + 1 (y) + 1 (o) = 8 banks
        g_pool = ctx.enter_context(tc.tile_pool(name="gps", bufs=1, space="PSUM"))
        z_pool = ctx.enter_context(tc.tile_pool(name="zps", bufs=1, space="PSUM"))
        y_pool = ctx.enter_context(tc.tile_pool(name="yps", bufs=1, space="PSUM"))
        o_pool = ctx.enter_context(tc.tile_pool(name="ops", bufs=1, space="PSUM"))

        ones_sb = const_pool.tile([128, 128], bf16)
        nc.vector.memset(ones_sb[:], 1.0)
        warm_sb = const_pool.tile([128, D_MODEL], bf16, name="warm_sb")
        nc.vector.memset(warm_sb[:], 1.0)
        warm = o_pool.tile([128, 512], f32, tag="o", name="warm")
        for wi in range(12):
            nc.tensor.matmul(warm[:], ones_sb[:], warm_sb[:],
                             start=True, stop=True)
        wv_sb = const_pool.tile([128, N_DT * H * D_V], bf16, name="wv_sb")
        wo_sb = const_pool.tile([128, HP * N_DT * 128], bf16, name="wo_sb")
        beff_sb = const_pool.tile([128, N_DT], f32, name="beff_sb")

        Q2 = NQ // 2

        def emit_group(b, half, gi, FQ, f0, a3_t, a12r_t):
            """One score group: broadcast multiply + exp for FQ f-positions."""
            a3b = a3_t[half][:]
            in0 = bass.AP(a3b.tensor, a3b.offset,
                          [a3b.ap[0], [0, FQ], [NQ, H], [2, Q2], [1, 2]])
            sc = sc_pool.tile([128, 4 * HQ], bf16, tag="sc",
                              name=f"sc_{b}_{half}_{gi}")
            scb = sc[:]
            out_ap = bass.AP(scb.tensor, scb.offset,
                             [scb.ap[0], [HQ, FQ], [NQ, H], [2, Q2], [1, 2]])
            a12b = a12r_t[half][:]
            in1 = bass.AP(a12b.tensor, a12b.offset + f0 * H * 2,
                          [a12b.ap[0], [H * 2, FQ], [2, H], [0, Q2], [1, 2]])
            nc.vector.tensor_mul(out_ap, in0, in1)
            et = et_pool.tile([128, 4 * HQ], bf16, tag="et",
                              name=f"et_{b}_{half}_{gi}")
            nc.scalar.activation(et[:, :FQ * HQ], sc[:, :FQ * HQ], Exp)
            return et

        def prologue(b):
            """Input DMAs + first two score groups for batch b — emitted ahead
            of the previous batch's epilogue so DVE/ACT stay primed across the
            batch transition and et for kt0..7 is ready at batch start."""
            a3_t = [a3_pool.tile([128, HQ], bf16, tag=f"a3_{hf}",
                                 name=f"a3_{b}_{hf}") for hf in range(2)]
            for hf in range(2):
                nc.sync.dma_start(a3_t[hf][:],
                                  att3_t.ap()[b, hf * 128:(hf + 1) * 128, :])
            a12r_t = []
            for hf in range(2):
                a12r = a12r_pool.tile([128, F * H * 2], bf16, tag=f"a12r_{hf}",
                                      name=f"a12r_{b}_{hf}")
                nc.sync.dma_start(a12r[:],
                                  att12_pair.ap()[b, hf * 128:(hf + 1) * 128, :])
                a12r_t.append(a12r)
            groups0 = [1, 1, 2, 4, 4, 4] if b == 0 else [4, 4, 4, 4]
            et0 = emit_group(b, 0, 0, groups0[0], 0, a3_t, a12r_t)
            et1 = emit_group(b, 0, 1, groups0[1], groups0[0], a3_t, a12r_t)
            return a3_t, a12r_t, groups0, [et0, et1]

        def ep_copies(pend):
            """Drain unnormalized G~ from PSUM: dt1/dt2 on ACT, dt3/dt0 on
            DVE. Ordered so the deferred-matmul flushes (dt1@kt4, dt2/3@kt8)
            and step1 (needs dt0 last) never stall."""
            g_all, gps = pend["g_all"], pend["gps"]
            nc.scalar.activation(g_all[:, 1 * HQ:2 * HQ], gps[1][:], Copy)
            nc.scalar.activation(g_all[:, 2 * HQ:3 * HQ], gps[2][:], Copy)
            nc.vector.tensor_copy(g_all[:, 3 * HQ:4 * HQ], gps[3][:])
            nc.vector.tensor_copy(g_all[:, 0 * HQ:1 * HQ], gps[0][:])

        def ep_z(pend):
            zps = z_pool.tile([128, HQ], f32, tag="z")
            nc.tensor.matmul(zps[:], ones_sb[:], pend["esum"][:],
                             start=True, stop=True)
            zb = zb_pool.tile([128, HQ], f32, tag="zb")
            nc.vector.reciprocal_approx_fast(zb[:], zps[:])
            pend["zb"] = zb

        def ep_step1(pend):
            """y[p, hp*128+j] += sum_din wv2[din, (dt,hp) block][p] *
            G~[din, (dt, hp) q-block pair][j].

            lhsT packs two heads' W_v columns (even head -> partitions 0:64,
            odd -> 64:128); rhs is both heads' q-blocks. The diagonal 64x64
            blocks of each [128,128] product are the packed Y; off-diagonal
            blocks are unused garbage. Full-partition matmuls, one
            accumulation group for the whole bank."""
            y = y_pool.tile([128, 512], f32, tag="y")
            g_all = pend["g_all"]
            for i, dt in enumerate((1, 2, 3, 0)):
                for hp in range(HP):
                    lhsT = wv_sb[:, (dt * HP + hp) * 128:(dt * HP + hp + 1) * 128]
                    rhs = g_all[:, dt * HQ + hp * 128: dt * HQ + (hp + 1) * 128]
                    nc.tensor.matmul(y[:, hp * 128:(hp + 1) * 128], lhsT, rhs,
                                     start=(i == 0 and hp == 0),
                                     stop=(i == 3 and hp == HP - 1))
            pend["y"] = y

        def ep_finish(pend):
            y, zb, b = pend["y"], pend["zb"], pend["b"]
            # normalize + select diagonal blocks:
            #   ysb[p<64,  hp*64+q] = y[p, hp*128 + q]      * 1/Z[2hp,   q]
            #   ysb[p>=64, hp*64+q] = y[p, hp*128 + 64 + q] * 1/Z[2hp+1, q]
            ysb = ysb_pool.tile([128, HP * NQ], bf16, tag="ysb")
            for lo in (0, 1):
                yv = y[lo * 64:(lo + 1) * 64]
                in0 = bass.AP(yv.tensor, yv.offset + lo * NQ,
                              [yv.ap[0], [2 * NQ, HP], [1, NQ]])
                zbs = zb[lo * 64:(lo + 1) * 64]
                in1 = bass.AP(zbs.tensor, zbs.offset + lo * NQ,
                              [zbs.ap[0], [2 * NQ, HP], [1, NQ]])
                ys = ysb[lo * 64:(lo + 1) * 64, :]
                o_ap = bass.AP(ys.tensor, ys.offset, [ys.ap[0], [NQ, HP], [1, NQ]])
                nc.vector.tensor_mul(o_ap, in0, in1)
            # step2: out[dmt*128+m, q] = sum_{hp,(h,dv)} Wo * Y
            o_t = o_pool.tile([128, 512], f32, tag="o")
            first = True
            for dmt in range(N_DT):
                for hp in range(HP):
                    lhsT = wo_sb[:, (hp * N_DT + dmt) * 128:
                                 (hp * N_DT + dmt + 1) * 128]
                    rhs = ysb[:, hp * NQ:(hp + 1) * NQ]
                    nc.tensor.matmul(o_t[:, dmt * NQ:(dmt + 1) * NQ], lhsT, rhs,
                                     start=first,
                                     stop=(dmt == N_DT - 1 and hp == HP - 1))
                    first = False
            # bias add (per-partition per-dmt, broadcast over q) + to SBUF
            osb = osb_pool.tile([128, N_DT * NQ], f32, tag="osb")
            ov = o_t[:, 0:N_DT * NQ]
            in0 = bass.AP(ov.tensor, ov.offset, [ov.ap[0], [NQ, N_DT], [1, NQ]])
            bb = beff_sb[:]
            in1 = bass.AP(bb.tensor, bb.offset, [bb.ap[0], [1, N_DT], [0, NQ]])
            oo = osb[:]
            o_ap = bass.AP(oo.tensor, oo.offset, [oo.ap[0], [NQ, N_DT], [1, NQ]])
            nc.vector.tensor_add(o_ap, in0, in1)
            oap = out.ap()
            dst = bass.AP(oap.tensor, oap.offset + b * NQ,
                          [[B_LOC * NQ, 128], [128 * B_LOC * NQ, N_DT], [1, NQ]])
            nc.sync.dma_start(dst, osb[:])

        FL = {1: 4, 2: 8, 3: 8}    # deferred-flush kt per d_in tile
        pro = prologue(0)
        pend = None
        for b in range(B_LOC):
            a3_t, a12r_t, groups0, pre_ets = pro
            gps = [g_pool.tile([128, HQ], f32, tag=f"g{dt}", name=f"g_{b}_{dt}",
                               bufs=(2 if dt == 0 else 1))
                   for dt in range(N_DT)]
            esum = esum_pool.tile([128, HQ], bf16, name=f"esum_{b}")
            g_all = gall_pool.tile([128, N_DT * HQ], bf16, tag="gall",
                                   name=f"gall_{b}")
            # third score group emitted before the copies so its exp is not
            # blocked behind them in the ACT FIFO
            et2 = emit_group(b, 0, 2, groups0[2], groups0[0] + groups0[1],
                             a3_t, a12r_t)
            pre_ets = pre_ets + [et2]
            if pend is not None:
                ep_copies(pend)
            deferred = {1: [], 2: [], 3: []}
            for half in range(2):
                groups = groups0 if half == 0 else [4, 4, 4, 4]
                f0 = 0
                for gi, FQ in enumerate(groups):
                    if half == 0 and gi < 3:
                        et = pre_ets[gi]
                    else:
                        et = emit_group(b, half, gi, FQ, f0, a3_t, a12r_t)
                    for j in range(FQ):
                        kt = half * F + f0 + j
                        vt = vt_pool.tile([128, D_IN], bf16, tag="vt",
                                          name=f"vt_{b}_{kt}")
                        nc.sync.dma_start(
                            vt[:], values_r.ap()[b, kt * 128:(kt + 1) * 128, :])
                        ets = et[:, j * HQ:(j + 1) * HQ]
                        last = (kt == N_KT - 1)
                        nc.tensor.matmul(gps[0][:], vt[:, 0:128], ets,
                                         start=(kt == 0), stop=last)
                        for dt in (1, 2, 3):
                            if kt < FL[dt]:
                                deferred[dt].append((vt, ets))
                            else:
                                if kt == FL[dt]:
                                    for i, (dvt, dets) in enumerate(deferred[dt]):
                                        nc.tensor.matmul(
                                            gps[dt][:],
                                            dvt[:, dt * 128:(dt + 1) * 128],
                                            dets, start=(i == 0), stop=False)
                                nc.tensor.matmul(gps[dt][:],
                                                 vt[:, dt * 128:(dt + 1) * 128],
                                                 ets, start=False, stop=last)
                        if kt == 0:
                            nc.vector.tensor_copy(esum[:], ets)
                        else:
                            nc.vector.tensor_add(esum[:], esum[:], ets)
                        if pend is not None:
                            if kt == 4:
                                ep_z(pend)
                            elif kt == 10:
                                ep_step1(pend)
                            elif kt == 12:
                                ep_finish(pend)
                    f0 += FQ
                if b == 0 and half == 0:
                    # projection weights, emitted mid-stream so the transfers
                    # never contend with critical prefetches
                    wq = N_DT * H * D_V // 2
                    for mi in range(2):
                        nc.sync.dma_start(wv_sb[:, mi * wq:(mi + 1) * wq],
                                          wv_all.ap()[:, mi * wq:(mi + 1) * wq])
                    oq = HP * N_DT * 128 // 2
                    for mi in range(2):
                        nc.sync.dma_start(wo_sb[:, mi * oq:(mi + 1) * oq],
                                          wo_all.ap()[:, mi * oq:(mi + 1) * oq])
                    nc.sync.dma_start(beff_sb[:], beff_t.ap())

            if b + 1 < B_LOC:
                pro = prologue(b + 1)
            pend = {"b": b, "gps": gps, "esum": esum, "g_all": g_all}

        ep_copies(pend)
        ep_z(pend)
        ep_step1(pend)
        ep_finish(pend)

    nc.compile()
    return nc


def _get_nc():
    if "nc" not in _NC_CACHE:
        _NC_CACHE["nc"] = _build_nc()
    return _NC_CACHE["nc"]


def _host_prep(att12, att3, values, W_v, b_v, W_o, b_o):
    att12 = np.asarray(att12, np.float32)
    att3 = np.asarray(att3, np.float32)
    values = np.asarray(values, np.float32)
    W_v = np.asarray(W_v, np.float32)
    b_v = np.asarray(b_v, np.float32)
    W_o = np.asarray(W_o, np.float32)
    b_o = np.asarray(b_o, np.float32)

    values_r = np.ascontiguousarray(values[:, _PERM, :]).astype(BF16)
    att3_t = np.ascontiguousarray(
        att3.transpose(0, 3, 1, 2).reshape(B, NCELL, HQ)).astype(BF16)
    att12_r = np.ascontiguousarray(
        att12.transpose(0, 1, 2, 4, 5, 3).reshape(B, NCELL, F * H)).astype(BF16)
    att12_pair = np.ascontiguousarray(np.broadcast_to(
        att12_r[:, :, :, None], (B, NCELL, F * H, 2)).reshape(
        B, NCELL, F * H * 2))

    # step1 weights: wv_all[p, (dt*HP+hp)*128 + lo*64 + dv]
    #              = W_v[(2hp+lo)*64+dv, dt*128+p]
    wv_all = np.ascontiguousarray(
        W_v.reshape(HP, 2, D_V, N_DT, 128).transpose(4, 3, 0, 1, 2)
        .reshape(128, N_DT * H * D_V)).astype(BF16)
    # step2 weights: wo_all[p, (hp*N_DT+dmt)*128+m] = W_o[dmt*128+m, h(p), dv(p)]
    Wo4 = W_o.reshape(N_DT, 128, H, D_V)
    wo = np.empty((128, HP, N_DT, 128), np.float32)
    for hp in range(HP):
        wo[0:64, hp] = Wo4[:, :, 2 * hp, :].transpose(2, 0, 1)
        wo[64:128, hp] = Wo4[:, :, 2 * hp + 1, :].transpose(2, 0, 1)
    wo_all = np.ascontiguousarray(
        wo.reshape(128, HP * N_DT * 128)).astype(BF16)

    b_eff = b_o + np.einsum("dhv,hv->d", W_o.reshape(D_MODEL, H, D_V),
                            b_v.reshape(H, D_V))
    beff_t = np.ascontiguousarray(
        b_eff.reshape(N_DT, 128).T).astype(np.float32)
    return values_r, att3_t, att12_pair, wv_all, wo_all, beff_t


def _in_maps(values_r, att3_t, att12_pair, wv_all, wo_all, beff_t):
    in_maps = []
    for core in range(N_CORES):
        s = slice(core * B_LOC, (core + 1) * B_LOC)
        in_maps.append({
            "values_r": np.ascontiguousarray(values_r[s]),
            "att3_t": np.ascontiguousarray(att3_t[s]),
            "att12_pair": np.ascontiguousarray(att12_pair[s]),
            "wv_all": wv_all,
            "wo_all": wo_all,
            "beff_t": beff_t,
        })
    return in_maps


def kernel(att12, att3, values, W_v, b_v, W_o, b_o):
    from concourse.bass_utils import run_bass_kernel_spmd

    prepped = _host_prep(att12, att3, values, W_v, b_v, W_o, b_o)
    in_maps = _in_maps(*prepped)

    nc = _get_nc()
    res = run_bass_kernel_spmd(nc, in_maps, core_ids=list(range(N_CORES)))
    # out[core] is [N_DT, 128, B_LOC*NQ]; final layout fix on host
    out = np.concatenate(
        [res.results[i]["out"].reshape(N_DT, 128, B_LOC * NQ)
         .transpose(2, 0, 1).reshape(B_LOC, NQ, D_MODEL)
         for i in range(N_CORES)], axis=0)
    return np.ascontiguousarray(out.astype(np.float32))
